# revision 57
# baseline (speedup 1.0000x reference)
"""TRN2 Bass kernel for nn_Attention_47665547051353.

Reference computation (B=4, C=512, N=2048, H=8, hd=64):
    qkv  = w_qkv @ x                           # 1x1 conv
    attn = softmax_j( k^T q * hd^-0.5 )        # softmax over QUERIES j
    out  = w_proj @ (v @ attn) + b_proj
Sharding (8 cores): core c -> batch b = c//2, head-group g = c%2 (4 heads).

Schedule: the exp stream on the Scalar (ACT) engine is the roofline
(128 x ~1.03us ACTIVATE + accumulator reads); everything else is
arranged to keep it saturated end-to-end:
  - head: PE warm-up spin during input DMAs (HAM at 2.4GHz before the
    first real matmul), x/weights split across sync+gpsimd DMA queues
    with per-chunk x tiles, prologue order k0h0 -> q0h0 -> q0h1 so unit
    0's scores hit PSUM as soon as x lands; the remaining QKV
    projections are spread over the first ~16 units' PE slack.
  - steady state: scores double-buffered in PSUM (2x[128,1024]) +
    per-head AV accumulator (4 banks); consecutive matmuls alternate
    PE row/col groups so LDWEIGHTS overlaps and K=64 pairs pack.
  - tail: the last head's A-tile copies split between Scalar+Vector
    (different PSUM banks); the output projection reuses the freed
    scores PSUM banks as [128,1024] half-blocks so its matmuls start
    the moment the last exp finishes, ordered done-heads-first; per
    half-block copy + DMA pipelined on rotating queues.
"""
import sys

if "/opt/trn_rl_repo" not in sys.path:
    sys.path.insert(0, "/opt/trn_rl_repo")

import numpy as np

import concourse.bass as bass
import concourse.tile as tile
import concourse.mybir as mybir
from concourse import bacc
from concourse.bass_utils import run_bass_kernel_spmd

F32 = mybir.dt.float32
F16 = mybir.dt.float16
EXP = mybir.ActivationFunctionType.Exp

B, C, N = 4, 512, 2048
H, HD = 8, 64
SCALE = HD ** -0.5
P = 128
CC = C // P          # 4 contraction chunks over channels
NT = N // P          # 16 key blocks
HG = H // 2          # 4 heads per core (one head-group)
N_CORES = 8

_CACHE = {}


def build_program(dbg=False):
    nc = bacc.Bacc("TRN2", target_bir_lowering=False, debug=False)
    x_ap = nc.dram_tensor("x", [C, N], F16, kind="ExternalInput").ap()
    # weights arrive host-pre-arranged partition-major so every DMA line
    # is a contiguous 2KB+ read (the naive [C, O] layout forces 512B
    # descriptors, ~5x slower, and the weight loads gate the whole head)
    wq_ap = nc.dram_tensor("wqT", [P, CC * HG * HD], F16, kind="ExternalInput").ap()
    wk_ap = nc.dram_tensor("wkT", [P, CC * HG * HD], F16, kind="ExternalInput").ap()
    wv_ap = nc.dram_tensor("wvT", [P, CC * HG * HD], F16, kind="ExternalInput").ap()
    wp_ap = nc.dram_tensor("wpT", [P, 4 * C], F16, kind="ExternalInput").ap()
    out_ap = nc.dram_tensor("out", [C, N], F32, kind="ExternalOutput").ap()

    with tile.TileContext(nc) as tc:
        with (
            tc.tile_pool(name="const", bufs=1) as const,
            tc.tile_pool(name="big", bufs=1) as big,
            tc.tile_pool(name="ppool", bufs=19) as ppool,
            tc.tile_pool(name="small", bufs=16) as small,
            tc.tile_pool(name="outp", bufs=4) as outp,
        ):
            # ACT exp-table preload (overlaps the input DMAs)
            warm = const.tile([P, 1], F32, tag="warm")
            warm2 = small.tile([P, 1], F32, tag="warm2")
            nc.vector.memset(warm, 0.0)
            nc.scalar.activation(warm2, warm, EXP)

            # scores pool; also reused by the output projection at the tail
            scps_cm = tc.tile_pool(name="scps", bufs=2, space="PSUM")
            scps = scps_cm.__enter__()

            QK = {}
            VT = big.tile([P, NT, HG * HD], F16)
            wp_r = const.tile([P, 4, C], F16)
            A = {}
            units = [(t, h, i) for t in range(2) for h in range(2)
                     for i in range(NT)]
            av_tiles = {}
            pending = []

            def emit_unit(t, h, i, mid=None):
                kt, qt = QK[("k", t)], QK[("q", t)]
                ktd, qtd = QK.get(("kd", t)), QK.get(("qd", t))
                p_t = ppool.tile([P, N], F16, tag="p")
                first_units = (t == 0 and h == 0 and i < 8)
                for half in range(2):
                    if half == 1 and mid is not None:
                        mid()
                    sps = scps.tile([P, 1024], F32, tag="s")
                    for jc in range(2):
                        # alternate PE row groups per matmul so each
                        # LDWEIGHTS overlaps the previous matmul (units 0-7
                        # skip it: the swapped duplicates aren't DMA'd yet
                        # and would stall the in-order PE)
                        if (i + jc) % 2 == 0 or first_units:
                            kk, qq, rb = kt, qt, h * HD
                        else:
                            kk, qq, rb = ktd, qtd, (1 - h) * HD
                        nc.tensor.matmul(
                            sps[:, jc * 512:(jc + 1) * 512],
                            kk[rb:rb + HD, i * P:(i + 1) * P],
                            qq[rb:rb + HD,
                               half * 1024 + jc * 512:half * 1024 + (jc + 1) * 512],
                            start=True, stop=True,
                        )
                    # half 0's row-sum is a DVE reduce over the exp'd fp16
                    # tile (tensor_reduce is ~1.4us/1024 — a full-row
                    # reduce would make DVE the bottleneck); half 1 uses
                    # the ACT accumulator (+182ns READ_ACC).  The add and
                    # reciprocal ride GpSimd (normalize_recip writes
                    # 1/denom back in place).
                    if half == 0:
                        nc.scalar.activation(
                            p_t[:, 0:1024], sps, EXP, scale=SCALE)
                        s0 = small.tile([P, 1], F32, tag="sum0")
                        nc.vector.reduce_sum(s0, p_t[:, 0:1024],
                                             axis=mybir.AxisListType.X)
                    else:
                        s1 = small.tile([P, 1], F32, tag="sum1")
                        nc.scalar.activation(
                            p_t[:, 1024:2048], sps,
                            EXP, scale=SCALE, accum_out=s1)
                s_all = small.tile([P, 1], F32, tag="stot")
                nc.vector.tensor_add(s_all, s0, s1)
                r_t = small.tile([P, 1], F32, tag="rcp")
                nc.vector.reciprocal(r_t, s_all)
                return p_t, r_t

            def emit_av(avps, t, h, i, p_t, r_t):
                vp = small.tile([P, HD], F16, tag="vp")
                hl = 2 * t + h
                nc.vector.tensor_scalar_mul(
                    vp, VT[:, i, hl * HD:(hl + 1) * HD], r_t)
                if (t, h) not in av_tiles:
                    av_new = avps.tile([P, N], F32, tag="av")
                    av_tiles[(t, h)] = av_new
                av = av_tiles[(t, h)]
                for jc4 in range(4):
                    # alternate output col groups per matmul; the halves
                    # are summed by the duplicated projection rows
                    par = (i + jc4) % 2
                    q0 = (par + jc4) % 2
                    nc.tensor.matmul(
                        av[par * HD:(par + 1) * HD,
                           jc4 * 512:(jc4 + 1) * 512],
                        vp,
                        p_t[:, jc4 * 512:(jc4 + 1) * 512],
                        start=(i == q0), stop=(i == NT - 2 + q0),
                        tile_position=(0, par * HD),
                        skip_group_check=True,
                    )
                if i == NT - 1:
                    a_h = big.tile([P, N], F16, tag=f"a{t}{h}")
                    av_done = av_tiles.pop((t, h))
                    last = (t == 1 and h == 1)
                    for q4 in range(4):
                        src = av_done[:, q4 * 512:(q4 + 1) * 512]
                        dst = a_h[:, q4 * 512:(q4 + 1) * 512]
                        if last and q4 < 2:
                            # ACT is idle at this point; DVE+ACT hit
                            # different PSUM banks in parallel
                            nc.scalar.copy(dst, src)
                        else:
                            nc.vector.tensor_copy(dst, src)
                    A[(t, h)] = a_h

            with tc.tile_pool(name="ld", bufs=1) as ld, \
                 tc.tile_pool(name="props", bufs=2, space="PSUM") as props:
                # ---- input DMAs on the sync+gpsimd queues (the scalar
                # HWDGE queue is measurably slower for bulk transfers);
                # gating weights first and t-split so only the pair-0
                # halves (2.25MB total with x) gate the first scores.
                # The head is aggregate-HBM-bound: all 8 cores pull their
                # inputs at once, so per-core BW is ~210GB/s, not 358. ----
                x_view = x_ap.rearrange("(cc p) n -> cc p n", p=P)
                x_r = [ld.tile([P, N], F16, tag=f"x{cc}", name=f"x_r{cc}")
                       for cc in range(CC)]
                wq_r = ld.tile([P, 2, CC, P], F16)
                wk_r = ld.tile([P, 2, CC, P], F16)
                wv_r = ld.tile([P, 2, CC, P], F16)
                wq_v = wq_ap.rearrange("p (t cc o) -> p t cc o", t=2, cc=CC)
                wk_v = wk_ap.rearrange("p (t cc o) -> p t cc o", t=2, cc=CC)
                wv_v = wv_ap.rearrange("p (t cc o) -> p t cc o", t=2, cc=CC)
                nc.sync.dma_start(out=wk_r[:, 0], in_=wk_v[:, 0])
                nc.gpsimd.dma_start(out=wq_r[:, 0], in_=wq_v[:, 0])
                nc.sync.dma_start(out=x_r[0], in_=x_view[0])
                nc.gpsimd.dma_start(out=x_r[1], in_=x_view[1])
                nc.sync.dma_start(out=x_r[2], in_=x_view[2])
                nc.gpsimd.dma_start(out=x_r[3], in_=x_view[3])
                nc.sync.dma_start(out=wk_r[:, 1], in_=wk_v[:, 1])
                nc.gpsimd.dma_start(out=wq_r[:, 1], in_=wq_v[:, 1])
                nc.gpsimd.dma_start(out=wv_r, in_=wv_v)

                # ---- PE warm-up: tiny matmuls ahead of the first real
                # matmul, plus bursts gated on each arriving x chunk (their
                # rhs reads x_r[cc], so each burst fires exactly when its
                # chunk lands) — the HAM stays un-throttled (2.4 GHz)
                # through the whole DMA window by construction.  The
                # scratch PSUM tile is the scores pool's first allocation:
                # spins finish before any unit reuses that buffer. ----
                spin_w = ld.tile([P, P], F16, tag="spin_w")
                nc.vector.memset(spin_w, 0.0)
                spin_ps = scps.tile([P, 1024], F32, tag="s", name="spin_ps")

                def spin(n, rhs=None):
                    r = spin_w[:, 0:P] if rhs is None else rhs
                    for _ in range(n):
                        nc.tensor.matmul(spin_ps[:, 0:P], spin_w, r,
                                         start=True, stop=True)

                spin(14)

                def w_chunk(w_r, t, cc):
                    return w_r[:, t, cc, :]

                def emit_qk_chunk(wname, w_r, t, half, cast_on=None):
                    """One [128,1024] output chunk of a q/k projection."""
                    key = (wname, t)
                    if key not in QK:
                        dst_new = big.tile([P, N], F16, tag=f"{wname}{t}")
                        QK[key] = dst_new
                    dst = QK[key]
                    ps = props.tile([P, 1024], F32, tag="qk")
                    for cc in range(CC):
                        for jc in range(2):
                            j0 = jc * 512
                            nc.tensor.matmul(
                                ps[:, j0:j0 + 512],
                                w_chunk(w_r, t, cc),
                                x_r[cc][:, half * 1024 + j0:half * 1024 + j0 + 512],
                                start=(cc == 0), stop=(cc == CC - 1),
                            )
                    dslice = dst[:, half * 1024:(half + 1) * 1024]
                    if cast_on is nc.scalar:
                        nc.scalar.copy(dslice, ps)
                    else:
                        nc.vector.tensor_copy(dslice, ps)
                    if half == 1:
                        dstd = big.tile([P, N], F16, tag=f"{wname}d{t}")
                        # dups ride gpsimd: the sync queue is busy with the
                        # VT transposes, and scalar-queue DMAs steal ACT
                        # time mid-stream
                        nc.gpsimd.dma_start(out=dstd[0:HD, :], in_=dst[HD:2 * HD, :])
                        nc.gpsimd.dma_start(out=dstd[HD:2 * HD, :], in_=dst[0:HD, :])
                        QK[(wname + "d", t)] = dstd

                # pair-0 K and Q interleaved cc-wise (two PSUM buffers) so
                # after the last x chunk lands only ~4 matmuls + the casts
                # remain before unit 0's scores; q0h0's cast runs on the
                # idle Scalar engine in parallel with k0h0's on Vector.
                k0_key, q0_key = ("k", 0), ("q", 0)
                QK[k0_key] = big.tile([P, N], F16, tag="k0", name="k0t")
                QK[q0_key] = big.tile([P, N], F16, tag="q0", name="q0t")
                ps_k = props.tile([P, 1024], F32, tag="qk", name="ps_k")
                ps_q = props.tile([P, 1024], F32, tag="qk", name="ps_q")
                for cc in range(CC):
                    if cc < CC - 1:
                        spin(8, rhs=x_r[cc][:, 0:P])
                    for ps, w_r in ((ps_k, wk_r), (ps_q, wq_r)):
                        for jc in range(2):
                            j0 = jc * 512
                            nc.tensor.matmul(
                                ps[:, j0:j0 + 512],
                                w_chunk(w_r, 0, cc),
                                x_r[cc][:, j0:j0 + 512],
                                start=(cc == 0), stop=(cc == CC - 1),
                            )
                nc.vector.tensor_copy(QK[k0_key][:, 0:1024], ps_k)
                nc.scalar.copy(QK[q0_key][:, 0:1024], ps_q)

                def emit_v_chunk(vt2, half, vr):
                    ps = props.tile([P, 1024], F32, tag="qk")
                    for cc in range(CC):
                        for jc in range(2):
                            j0 = jc * 512
                            nc.tensor.matmul(
                                ps[:, j0:j0 + 512],
                                w_chunk(wv_r, vt2, cc),
                                x_r[cc][:, half * 1024 + j0:half * 1024 + j0 + 512],
                                start=(cc == 0), stop=(cc == CC - 1),
                            )
                    nc.vector.tensor_copy(vr[:, half * 1024:(half + 1) * 1024], ps)
                    if half == 1:
                        # transpose DMAs are HWDGE-only; keep them all on
                        # the sync queue (plenty of slack before first use)
                        for nt in range(NT):
                            nc.sync.dma_start(
                                out=VT[:, nt, vt2 * P:(vt2 + 1) * P],
                                in_=vr[:, nt * P:(nt + 1) * P],
                                transpose=True,
                            )

                # v projections / VT transposes / pair-1 Q/K spread over
                # every other unit of the first 17 (one fill chunk is
                # ~1.7us of PE time vs ~2.2us of ACT time per unit, so
                # half a fill per unit keeps the exp stream PE-limited
                # never ACT-starved); their AVs drain afterwards at
                # ~1.5/unit.  Deadlines: k0h1 before unit 8's QK, q1/k1
                # dups before unit 32's QK (~t94us), VT vt2=1 before the
                # first pair-1 AV drain (~t112us).
                vrow0 = ld.tile([P, N], F16, tag="vrow0")
                vrow1 = ld.tile([P, N], F16, tag="vrow1")
                fill_at = {
                    1: lambda: emit_qk_chunk("k", wk_r, 0, 1),
                    2: lambda: emit_v_chunk(0, 0, vrow0),
                    4: lambda: emit_v_chunk(0, 1, vrow0),
                    6: lambda: emit_qk_chunk("q", wq_r, 1, 0),
                    8: lambda: emit_qk_chunk("q", wq_r, 1, 1),
                    10: lambda: emit_qk_chunk("k", wk_r, 1, 0),
                    12: lambda: emit_qk_chunk("k", wk_r, 1, 1),
                    14: lambda: emit_v_chunk(1, 0, vrow1),
                    16: lambda: emit_v_chunk(1, 1, vrow1),
                }
                n_pre = 17

                # wp is only needed by the tail projection; keep it off the
                # critical input-bandwidth window
                nc.gpsimd.dma_start(out=wp_r, in_=wp_ap.rearrange("p (t o) -> p t o", t=4))

                # unit 0's half-0 scores run as soon as k0h0+q0h0 exist;
                # q0h1 is projected between its two halves so the first
                # ACTIVATE fires while q0's second half still computes
                for g in range(n_pre):
                    u = units[g]
                    if g == 0:
                        mid = lambda: emit_qk_chunk("q", wq_r, 0, 1)
                    else:
                        mid = None
                    pending.append((u, emit_unit(*u, mid=mid)))
                    if g in fill_at:
                        fill_at[g]()

            # ---- main attention stream (software-pipelined) ----
            with tc.tile_pool(name="avps", bufs=1, space="PSUM") as avps:
                for g in range(n_pre, len(units)):
                    u = units[g]
                    pending.append((u, emit_unit(*u)))
                    # drain the fill-region backlog at ~1.5 AVs per unit
                    # (PE stays under the ACT period), down to a floor of
                    # 3 (~7us of exp runway) so the head-transition A-tile
                    # copies never starve the ACT stream
                    drain_to = max(3, n_pre - (g - n_pre + 1) // 2 * 3 + 2)
                    while len(pending) > drain_to:
                        (pt_, ph_, pi_), (p_t, r_t) = pending.pop(0)
                        emit_av(avps, pt_, ph_, pi_, p_t, r_t)
                while pending:
                    (pt_, ph_, pi_), (p_t, r_t) = pending.pop(0)
                    emit_av(avps, pt_, ph_, pi_, p_t, r_t)

            # ---- output projection, in a 3-deep PSUM pool over the banks
            # freed by scores+AV (its first tile lands on the ex-scores
            # banks, so the matmuls start the moment the last exp
            # finishes); hi emitted done-heads-first so the in-order PE
            # only waits on the last head at the very end; hi-cast on the
            # idle Scalar engine, fp16 residual on Vector ----
            out_q = [nc.sync, nc.gpsimd]
            with tc.tile_pool(name="prx", bufs=2, space="PSUM") as prx:
                def proj_fill(blk):
                    """hi 0-2 matmuls of a block (A(1,1)-independent).
                    Blocks 0-1 use the scores pool's buffers (free the
                    moment the last exp ends — so the PE rolls straight
                    from attention into the projection and stays warm);
                    later blocks rotate through the ex-AV banks."""
                    ot, hf = blk // 2, blk % 2
                    if blk % 4 < 2:
                        pso = scps.tile([P, 1024], F32, tag="s")
                    else:
                        pso = prx.tile([P, 1024], F32, tag="pr")
                    base = hf * 1024
                    for hi in range(3):
                        t2, h2 = hi // 2, hi % 2
                        for jc in range(2):
                            nc.tensor.matmul(
                                pso[:, jc * 512:(jc + 1) * 512],
                                wp_r[:, hi, ot * P:(ot + 1) * P],
                                A[(t2, h2)][:, base + jc * 512:base + (jc + 1) * 512],
                                start=(hi == 0), stop=False,
                            )
                    return pso

                def proj_retire(blk, pso):
                    """last-head matmuls + split-float copies + DMAs."""
                    ot, hf = blk // 2, blk % 2
                    base = hf * 1024
                    for jc in range(2):
                        nc.tensor.matmul(
                            pso[:, jc * 512:(jc + 1) * 512],
                            wp_r[:, 3, ot * P:(ot + 1) * P],
                            A[(1, 1)][:, base + jc * 512:base + (jc + 1) * 512],
                            start=False, stop=True,
                        )
                    o_sb = outp.tile([P, 1024], F32, tag="o")
                    # half-granular copy+DMA: transfers start ~0.7us
                    # earlier and stream on both queues concurrently
                    for q4 in range(2):
                        sl = slice(q4 * 512, (q4 + 1) * 512)
                        nc.vector.tensor_copy(o_sb[:, sl], pso[:, sl])
                        out_q[(blk + q4) % 2].dma_start(
                            out=out_ap[ot * P:(ot + 1) * P,
                                       base + q4 * 512:base + (q4 + 1) * 512],
                            in_=o_sb[:, sl])

                # 3-block software pipeline: the first blocks' hi 0-2
                # matmuls bridge the A(1,1) copy latency so the PE never
                # idles (and never HAM-re-throttles) through the tail
                inflight = []
                for blk in range(8):
                    inflight.append((blk, proj_fill(blk)))
                    if len(inflight) == 3:
                        proj_retire(*inflight.pop(0))
                while inflight:
                    proj_retire(*inflight.pop(0))

            scps_cm.__exit__(None, None, None)

    nc.compile()
    return nc


def _prearrange(wT, chunks):
    """[chunks*128, O] -> [128, chunks*O]: partition-major layout so every
    DMA line is one contiguous read per partition."""
    O = wT.shape[1]
    return np.ascontiguousarray(
        wT.reshape(chunks, P, O).transpose(1, 0, 2).reshape(P, chunks * O))


def _prearrange_qkv(wT):
    """[C, 256] -> [128, 2*CC*128]: partition-major AND t-block-major so
    the pair-0 half can be DMA'd alone with contiguous lines."""
    return np.ascontiguousarray(
        wT.reshape(CC, P, 2, P).transpose(1, 2, 0, 3).reshape(P, 2 * CC * P))


def _shard_weights(w_qkv, w_proj):
    """Per head-group g: transposed q/k/v weight shards (logically [C, 256]
    with output column order o = 64*h_local + d, stored partition-major)
    and projection shard (logically [256->dup 512, C], partition-major)."""
    shards = []
    for g in range(2):
        heads = range(HG * g, HG * (g + 1))
        q_rows = [h * 3 * HD + d for h in heads for d in range(HD)]
        k_rows = [h * 3 * HD + HD + d for h in heads for d in range(HD)]
        v_rows = [h * 3 * HD + 2 * HD + d for h in heads for d in range(HD)]
        a_chans = [h * HD + (r % HD) for h in heads for r in range(P)]
        shards.append({
            "wqT": _prearrange_qkv(np.ascontiguousarray(w_qkv[q_rows, :].T)),
            "wkT": _prearrange_qkv(np.ascontiguousarray(w_qkv[k_rows, :].T)),
            "wvT": _prearrange_qkv(np.ascontiguousarray(w_qkv[v_rows, :].T)),
            "wpT": _prearrange(np.ascontiguousarray(w_proj[:, a_chans].T), 4),
        })
    return shards


def kernel(x, w_qkv, w_proj, b_proj, _trace=False, _trace_kwargs=None):
    x = np.asarray(x, dtype=np.float32)
    w_qkv = np.asarray(w_qkv, dtype=np.float32)
    w_proj = np.asarray(w_proj, dtype=np.float32)
    b_proj = np.asarray(b_proj, dtype=np.float32)

    if "nc" not in _CACHE:
        _CACHE["nc"] = build_program()
    nc = _CACHE["nc"]

    shards = _shard_weights(w_qkv, w_proj)
    shards = [{k: v.astype(np.float16) for k, v in s.items()} for s in shards]
    in_maps = []
    for core in range(N_CORES):
        b, g = core // 2, core % 2
        m = {"x": np.ascontiguousarray(x[b].astype(np.float16))}
        m.update(shards[g])
        in_maps.append(m)

    kw = {}
    if _trace:
        kw.update(trace=True, trace_cores=[0], **(_trace_kwargs or {}))
    res = run_bass_kernel_spmd(nc, in_maps, list(range(N_CORES)), **kw)

    out = np.empty((B, C, N), dtype=np.float32)
    for b in range(B):
        out[b] = (res.results[2 * b]["out"] + res.results[2 * b + 1]["out"]
                  + b_proj[:, None])
    if _trace:
        _CACHE["last_result"] = res
    return out


# revision 58
# speedup vs baseline: 1.0099x; 1.0099x over previous
"""TRN2 Bass kernel for nn_Attention_47665547051353.

Reference computation (B=4, C=512, N=2048, H=8, hd=64):
    qkv  = w_qkv @ x                           # 1x1 conv
    attn = softmax_j( k^T q * hd^-0.5 )        # softmax over QUERIES j
    out  = w_proj @ (v @ attn) + b_proj
Sharding (8 cores): core c -> batch b = c//2, head-group g = c%2 (4 heads).

Schedule: the exp stream on the Scalar (ACT) engine is the roofline
(128 x ~1.02us ACTIVATE + 64 accumulator reads ~= 143us); everything
else is arranged to keep it saturated end-to-end (~199us @ full clock):
  - head (aggregate-HBM-bound: 8 cores pull inputs at once, ~210GB/s
    each): weights host-pre-arranged partition-major for contiguous DMA
    lines, t-split so only pair-0 halves gate the start; x per-chunk
    tiles on the sync+gpsimd queues (the scalar HWDGE queue is slower);
    PE warm-up matmuls gated on each arriving x chunk keep the HAM at
    2.4GHz through the DMA window; k0/q0 projections interleaved
    cc-wise so unit 0's scores hit PSUM ~1.5us after the last x chunk.
  - steady state: scores double-buffered in PSUM (2x[128,1024]) +
    per-head AV accumulator (4 banks); consecutive matmuls alternate
    PE row/col groups so LDWEIGHTS overlaps and K=64 pairs pack; half-0
    row-sums on a DVE reduce, half-1 on the ACT accumulator (halves
    READ_ACCUMULATOR time; a full DVE offload makes DVE the bottleneck
    and any GpSimd op in the chain adds ~5-7us semaphore latency);
    remaining QKV projections fill every other of the first 17 units,
    their AVs drain afterwards at ~1.5/unit down to a floor of 3.
  - tail (out-BW-bound, 4MB f32/core): the output projection is
    software-pipelined [fill hi0-2 | retire hi3+copy+DMA] across 4 PSUM
    tiles, the first two on the scores banks so the PE rolls straight
    from the last exp into projection matmuls without HAM re-throttle;
    the last head's A-tile copies split Scalar/Vector on different
    banks; half-block copies + DMAs stream on both queues.
"""
import sys

if "/opt/trn_rl_repo" not in sys.path:
    sys.path.insert(0, "/opt/trn_rl_repo")

import numpy as np

import concourse.bass as bass
import concourse.tile as tile
import concourse.mybir as mybir
from concourse import bacc
from concourse.bass_utils import run_bass_kernel_spmd

F32 = mybir.dt.float32
F16 = mybir.dt.float16
EXP = mybir.ActivationFunctionType.Exp

B, C, N = 4, 512, 2048
H, HD = 8, 64
SCALE = HD ** -0.5
P = 128
CC = C // P          # 4 contraction chunks over channels
NT = N // P          # 16 key blocks
HG = H // 2          # 4 heads per core (one head-group)
N_CORES = 8

_CACHE = {}


def build_program(dbg=False):
    nc = bacc.Bacc("TRN2", target_bir_lowering=False, debug=False)
    x_ap = nc.dram_tensor("x", [C, N], F16, kind="ExternalInput").ap()
    # weights arrive host-pre-arranged partition-major so every DMA line
    # is a contiguous 2KB+ read (the naive [C, O] layout forces 512B
    # descriptors, ~5x slower, and the weight loads gate the whole head)
    wq_ap = nc.dram_tensor("wqT", [P, CC * HG * HD], F16, kind="ExternalInput").ap()
    wk_ap = nc.dram_tensor("wkT", [P, CC * HG * HD], F16, kind="ExternalInput").ap()
    wv_ap = nc.dram_tensor("wvT", [P, CC * HG * HD], F16, kind="ExternalInput").ap()
    wp_ap = nc.dram_tensor("wpT", [P, 4 * C], F16, kind="ExternalInput").ap()
    out_ap = nc.dram_tensor("out", [C, N], F32, kind="ExternalOutput").ap()

    with tile.TileContext(nc) as tc:
        with (
            tc.tile_pool(name="const", bufs=1) as const,
            tc.tile_pool(name="big", bufs=1) as big,
            tc.tile_pool(name="ppool", bufs=19) as ppool,
            tc.tile_pool(name="small", bufs=16) as small,
            tc.tile_pool(name="outp", bufs=4) as outp,
        ):
            # ACT exp-table preload (overlaps the input DMAs)
            warm = const.tile([P, 1], F32, tag="warm")
            warm2 = small.tile([P, 1], F32, tag="warm2")
            nc.vector.memset(warm, 0.0)
            nc.scalar.activation(warm2, warm, EXP)

            # scores pool; also reused by the output projection at the tail
            scps_cm = tc.tile_pool(name="scps", bufs=2, space="PSUM")
            scps = scps_cm.__enter__()

            QK = {}
            VT = big.tile([P, NT, HG * HD], F16)
            wp_r = const.tile([P, 4, C], F16)
            A = {}
            units = [(t, h, i) for t in range(2) for h in range(2)
                     for i in range(NT)]
            av_tiles = {}
            pending = []

            def emit_unit(t, h, i, mid=None):
                kt, qt = QK[("k", t)], QK[("q", t)]
                ktd, qtd = QK.get(("kd", t)), QK.get(("qd", t))
                p_t = ppool.tile([P, N], F16, tag="p")
                first_units = (t == 0 and h == 0 and i < 8)
                for half in range(2):
                    if half == 1 and mid is not None:
                        mid()
                    sps = scps.tile([P, 1024], F32, tag="s")
                    for jc in range(2):
                        # alternate PE row groups per matmul so each
                        # LDWEIGHTS overlaps the previous matmul (units 0-7
                        # skip it: the swapped duplicates aren't DMA'd yet
                        # and would stall the in-order PE)
                        if (i + jc) % 2 == 0 or first_units:
                            kk, qq, rb = kt, qt, h * HD
                        else:
                            kk, qq, rb = ktd, qtd, (1 - h) * HD
                        nc.tensor.matmul(
                            sps[:, jc * 512:(jc + 1) * 512],
                            kk[rb:rb + HD, i * P:(i + 1) * P],
                            qq[rb:rb + HD,
                               half * 1024 + jc * 512:half * 1024 + (jc + 1) * 512],
                            start=True, stop=True,
                        )
                    # half 0's row-sum is a DVE reduce over the exp'd fp16
                    # tile (tensor_reduce is ~1.4us/1024 — a full-row
                    # reduce would make DVE the bottleneck); half 1 uses
                    # the ACT accumulator (+182ns READ_ACC).  The add and
                    # reciprocal ride GpSimd (normalize_recip writes
                    # 1/denom back in place).
                    if half == 0:
                        nc.scalar.activation(
                            p_t[:, 0:1024], sps, EXP, scale=SCALE)
                        s0 = small.tile([P, 1], F32, tag="sum0")
                        nc.vector.reduce_sum(s0, p_t[:, 0:1024],
                                             axis=mybir.AxisListType.X)
                    else:
                        s1 = small.tile([P, 1], F32, tag="sum1")
                        nc.scalar.activation(
                            p_t[:, 1024:2048], sps,
                            EXP, scale=SCALE, accum_out=s1)
                s_all = small.tile([P, 1], F32, tag="stot")
                nc.vector.tensor_add(s_all, s0, s1)
                r_t = small.tile([P, 1], F32, tag="rcp")
                nc.vector.reciprocal(r_t, s_all)
                return p_t, r_t

            def emit_av(avps, t, h, i, p_t, r_t):
                vp = small.tile([P, HD], F16, tag="vp")
                hl = 2 * t + h
                nc.vector.tensor_scalar_mul(
                    vp, VT[:, i, hl * HD:(hl + 1) * HD], r_t)
                if (t, h) not in av_tiles:
                    av_new = avps.tile([P, N], F32, tag="av")
                    av_tiles[(t, h)] = av_new
                av = av_tiles[(t, h)]
                for jc4 in range(4):
                    # alternate output col groups per matmul; the halves
                    # are summed by the duplicated projection rows
                    par = (i + jc4) % 2
                    q0 = (par + jc4) % 2
                    nc.tensor.matmul(
                        av[par * HD:(par + 1) * HD,
                           jc4 * 512:(jc4 + 1) * 512],
                        vp,
                        p_t[:, jc4 * 512:(jc4 + 1) * 512],
                        start=(i == q0), stop=(i == NT - 2 + q0),
                        tile_position=(0, par * HD),
                        skip_group_check=True,
                    )
                if i == NT - 1:
                    a_h = big.tile([P, N], F16, tag=f"a{t}{h}")
                    av_done = av_tiles.pop((t, h))
                    last = (t == 1 and h == 1)
                    for q4 in range(4):
                        src = av_done[:, q4 * 512:(q4 + 1) * 512]
                        dst = a_h[:, q4 * 512:(q4 + 1) * 512]
                        if last and q4 < 2:
                            # ACT is idle at this point; DVE+ACT hit
                            # different PSUM banks in parallel
                            nc.scalar.copy(dst, src)
                        else:
                            nc.vector.tensor_copy(dst, src)
                    A[(t, h)] = a_h

            with tc.tile_pool(name="ld", bufs=1) as ld, \
                 tc.tile_pool(name="props", bufs=2, space="PSUM") as props:
                # ---- input DMAs on the sync+gpsimd queues (the scalar
                # HWDGE queue is measurably slower for bulk transfers);
                # gating weights first and t-split so only the pair-0
                # halves (2.25MB total with x) gate the first scores.
                # The head is aggregate-HBM-bound: all 8 cores pull their
                # inputs at once, so per-core BW is ~210GB/s, not 358. ----
                x_view = x_ap.rearrange("(cc p) n -> cc p n", p=P)
                x_r = [ld.tile([P, N], F16, tag=f"x{cc}", name=f"x_r{cc}")
                       for cc in range(CC)]
                wq_r = ld.tile([P, 2, CC, P], F16)
                wk_r = ld.tile([P, 2, CC, P], F16)
                wv_r = ld.tile([P, 2, CC, P], F16)
                wq_v = wq_ap.rearrange("p (t cc o) -> p t cc o", t=2, cc=CC)
                wk_v = wk_ap.rearrange("p (t cc o) -> p t cc o", t=2, cc=CC)
                wv_v = wv_ap.rearrange("p (t cc o) -> p t cc o", t=2, cc=CC)
                nc.sync.dma_start(out=wk_r[:, 0], in_=wk_v[:, 0])
                nc.gpsimd.dma_start(out=wq_r[:, 0], in_=wq_v[:, 0])
                nc.sync.dma_start(out=x_r[0], in_=x_view[0])
                nc.gpsimd.dma_start(out=x_r[1], in_=x_view[1])
                nc.sync.dma_start(out=x_r[2], in_=x_view[2])
                nc.gpsimd.dma_start(out=x_r[3], in_=x_view[3])
                nc.sync.dma_start(out=wk_r[:, 1], in_=wk_v[:, 1])
                nc.gpsimd.dma_start(out=wq_r[:, 1], in_=wq_v[:, 1])
                nc.gpsimd.dma_start(out=wv_r, in_=wv_v)

                # ---- PE warm-up: tiny matmuls ahead of the first real
                # matmul, plus bursts gated on each arriving x chunk (their
                # rhs reads x_r[cc], so each burst fires exactly when its
                # chunk lands) — the HAM stays un-throttled (2.4 GHz)
                # through the whole DMA window by construction.  The
                # scratch PSUM tile is the scores pool's first allocation:
                # spins finish before any unit reuses that buffer. ----
                spin_w = ld.tile([P, P], F16, tag="spin_w")
                nc.vector.memset(spin_w, 0.0)
                spin_ps = scps.tile([P, 1024], F32, tag="s", name="spin_ps")

                def spin(n, rhs=None):
                    r = spin_w[:, 0:P] if rhs is None else rhs
                    for _ in range(n):
                        nc.tensor.matmul(spin_ps[:, 0:P], spin_w, r,
                                         start=True, stop=True)

                spin(14)

                def w_chunk(w_r, t, cc):
                    return w_r[:, t, cc, :]

                def emit_qk_chunk(wname, w_r, t, half, cast_on=None):
                    """One [128,1024] output chunk of a q/k projection."""
                    key = (wname, t)
                    if key not in QK:
                        dst_new = big.tile([P, N], F16, tag=f"{wname}{t}")
                        QK[key] = dst_new
                    dst = QK[key]
                    ps = props.tile([P, 1024], F32, tag="qk")
                    for cc in range(CC):
                        for jc in range(2):
                            j0 = jc * 512
                            nc.tensor.matmul(
                                ps[:, j0:j0 + 512],
                                w_chunk(w_r, t, cc),
                                x_r[cc][:, half * 1024 + j0:half * 1024 + j0 + 512],
                                start=(cc == 0), stop=(cc == CC - 1),
                            )
                    dslice = dst[:, half * 1024:(half + 1) * 1024]
                    if cast_on is nc.scalar:
                        nc.scalar.copy(dslice, ps)
                    else:
                        nc.vector.tensor_copy(dslice, ps)
                    if half == 1:
                        dstd = big.tile([P, N], F16, tag=f"{wname}d{t}")
                        # dups ride gpsimd: the sync queue is busy with the
                        # VT transposes, and scalar-queue DMAs steal ACT
                        # time mid-stream
                        nc.gpsimd.dma_start(out=dstd[0:HD, :], in_=dst[HD:2 * HD, :])
                        nc.gpsimd.dma_start(out=dstd[HD:2 * HD, :], in_=dst[0:HD, :])
                        QK[(wname + "d", t)] = dstd

                # pair-0 K and Q interleaved cc-wise (two PSUM buffers) so
                # after the last x chunk lands only ~4 matmuls + the casts
                # remain before unit 0's scores; q0h0's cast runs on the
                # idle Scalar engine in parallel with k0h0's on Vector.
                k0_key, q0_key = ("k", 0), ("q", 0)
                QK[k0_key] = big.tile([P, N], F16, tag="k0", name="k0t")
                QK[q0_key] = big.tile([P, N], F16, tag="q0", name="q0t")
                ps_k = props.tile([P, 1024], F32, tag="qk", name="ps_k")
                ps_q = props.tile([P, 1024], F32, tag="qk", name="ps_q")
                for cc in range(CC):
                    if cc < CC - 1:
                        spin(8, rhs=x_r[cc][:, 0:P])
                    for ps, w_r in ((ps_k, wk_r), (ps_q, wq_r)):
                        for jc in range(2):
                            j0 = jc * 512
                            nc.tensor.matmul(
                                ps[:, j0:j0 + 512],
                                w_chunk(w_r, 0, cc),
                                x_r[cc][:, j0:j0 + 512],
                                start=(cc == 0), stop=(cc == CC - 1),
                            )
                nc.vector.tensor_copy(QK[k0_key][:, 0:1024], ps_k)
                nc.scalar.copy(QK[q0_key][:, 0:1024], ps_q)

                def emit_v_chunk(vt2, half, vr):
                    ps = props.tile([P, 1024], F32, tag="qk")
                    for cc in range(CC):
                        for jc in range(2):
                            j0 = jc * 512
                            nc.tensor.matmul(
                                ps[:, j0:j0 + 512],
                                w_chunk(wv_r, vt2, cc),
                                x_r[cc][:, half * 1024 + j0:half * 1024 + j0 + 512],
                                start=(cc == 0), stop=(cc == CC - 1),
                            )
                    nc.vector.tensor_copy(vr[:, half * 1024:(half + 1) * 1024], ps)
                    if half == 1:
                        # transpose DMAs are HWDGE-only; keep them all on
                        # the sync queue (plenty of slack before first use)
                        for nt in range(NT):
                            nc.sync.dma_start(
                                out=VT[:, nt, vt2 * P:(vt2 + 1) * P],
                                in_=vr[:, nt * P:(nt + 1) * P],
                                transpose=True,
                            )

                # v projections / VT transposes / pair-1 Q/K spread over
                # every other unit of the first 17 (one fill chunk is
                # ~1.7us of PE time vs ~2.2us of ACT time per unit, so
                # half a fill per unit keeps the exp stream PE-limited
                # never ACT-starved); their AVs drain afterwards at
                # ~1.5/unit.  Deadlines: k0h1 before unit 8's QK, q1/k1
                # dups before unit 32's QK (~t94us), VT vt2=1 before the
                # first pair-1 AV drain (~t112us).
                vrow0 = ld.tile([P, N], F16, tag="vrow0")
                vrow1 = ld.tile([P, N], F16, tag="vrow1")
                fill_at = {
                    1: lambda: emit_qk_chunk("k", wk_r, 0, 1),
                    2: lambda: emit_v_chunk(0, 0, vrow0),
                    4: lambda: emit_v_chunk(0, 1, vrow0),
                    6: lambda: emit_qk_chunk("q", wq_r, 1, 0),
                    8: lambda: emit_qk_chunk("q", wq_r, 1, 1),
                    10: lambda: emit_qk_chunk("k", wk_r, 1, 0),
                    12: lambda: emit_qk_chunk("k", wk_r, 1, 1),
                    14: lambda: emit_v_chunk(1, 0, vrow1),
                    16: lambda: emit_v_chunk(1, 1, vrow1),
                }
                n_pre = 17

                # wp is only needed by the tail projection; keep it off the
                # critical input-bandwidth window
                nc.gpsimd.dma_start(out=wp_r, in_=wp_ap.rearrange("p (t o) -> p t o", t=4))

                # unit 0's half-0 scores run as soon as k0h0+q0h0 exist;
                # q0h1 is projected between its two halves so the first
                # ACTIVATE fires while q0's second half still computes
                for g in range(n_pre):
                    u = units[g]
                    if g == 0:
                        mid = lambda: emit_qk_chunk("q", wq_r, 0, 1)
                    else:
                        mid = None
                    pending.append((u, emit_unit(*u, mid=mid)))
                    if g in fill_at:
                        fill_at[g]()

            # ---- main attention stream (software-pipelined) ----
            with tc.tile_pool(name="avps", bufs=1, space="PSUM") as avps:
                for g in range(n_pre, len(units)):
                    u = units[g]
                    pending.append((u, emit_unit(*u)))
                    # drain the fill-region backlog at ~1.5 AVs per unit
                    # (PE stays under the ACT period), down to a floor of
                    # 3 (~7us of exp runway) so the head-transition A-tile
                    # copies never starve the ACT stream
                    drain_to = max(3, n_pre - (g - n_pre + 1) // 2 * 3 + 2)
                    while len(pending) > drain_to:
                        (pt_, ph_, pi_), (p_t, r_t) = pending.pop(0)
                        emit_av(avps, pt_, ph_, pi_, p_t, r_t)
                while pending:
                    (pt_, ph_, pi_), (p_t, r_t) = pending.pop(0)
                    emit_av(avps, pt_, ph_, pi_, p_t, r_t)

            # ---- output projection, in a 3-deep PSUM pool over the banks
            # freed by scores+AV (its first tile lands on the ex-scores
            # banks, so the matmuls start the moment the last exp
            # finishes); hi emitted done-heads-first so the in-order PE
            # only waits on the last head at the very end; hi-cast on the
            # idle Scalar engine, fp16 residual on Vector ----
            out_q = [nc.sync, nc.gpsimd]
            with tc.tile_pool(name="prx", bufs=2, space="PSUM") as prx:
                def proj_fill(blk):
                    """hi 0-2 matmuls of a block (A(1,1)-independent).
                    Blocks 0-1 use the scores pool's buffers (free the
                    moment the last exp ends — so the PE rolls straight
                    from attention into the projection and stays warm);
                    later blocks rotate through the ex-AV banks."""
                    ot, hf = blk // 2, blk % 2
                    if blk % 4 < 2:
                        pso = scps.tile([P, 1024], F32, tag="s")
                    else:
                        pso = prx.tile([P, 1024], F32, tag="pr")
                    base = hf * 1024
                    for hi in range(3):
                        t2, h2 = hi // 2, hi % 2
                        for jc in range(2):
                            nc.tensor.matmul(
                                pso[:, jc * 512:(jc + 1) * 512],
                                wp_r[:, hi, ot * P:(ot + 1) * P],
                                A[(t2, h2)][:, base + jc * 512:base + (jc + 1) * 512],
                                start=(hi == 0), stop=False,
                            )
                    return pso

                def proj_retire(blk, pso):
                    """last-head matmuls + split-float copies + DMAs."""
                    ot, hf = blk // 2, blk % 2
                    base = hf * 1024
                    for jc in range(2):
                        nc.tensor.matmul(
                            pso[:, jc * 512:(jc + 1) * 512],
                            wp_r[:, 3, ot * P:(ot + 1) * P],
                            A[(1, 1)][:, base + jc * 512:base + (jc + 1) * 512],
                            start=False, stop=True,
                        )
                    o_sb = outp.tile([P, 1024], F32, tag="o")
                    # half-granular copy+DMA: transfers start ~0.7us
                    # earlier and stream on both queues concurrently
                    for q4 in range(2):
                        sl = slice(q4 * 512, (q4 + 1) * 512)
                        nc.vector.tensor_copy(o_sb[:, sl], pso[:, sl])
                        out_q[(blk + q4) % 2].dma_start(
                            out=out_ap[ot * P:(ot + 1) * P,
                                       base + q4 * 512:base + (q4 + 1) * 512],
                            in_=o_sb[:, sl])

                # 3-block software pipeline: the first blocks' hi 0-2
                # matmuls bridge the A(1,1) copy latency so the PE never
                # idles (and never HAM-re-throttles) through the tail
                inflight = []
                for blk in range(8):
                    inflight.append((blk, proj_fill(blk)))
                    if len(inflight) == 3:
                        proj_retire(*inflight.pop(0))
                while inflight:
                    proj_retire(*inflight.pop(0))

            scps_cm.__exit__(None, None, None)

    nc.compile()
    return nc


def _prearrange(wT, chunks):
    """[chunks*128, O] -> [128, chunks*O]: partition-major layout so every
    DMA line is one contiguous read per partition."""
    O = wT.shape[1]
    return np.ascontiguousarray(
        wT.reshape(chunks, P, O).transpose(1, 0, 2).reshape(P, chunks * O))


def _prearrange_qkv(wT):
    """[C, 256] -> [128, 2*CC*128]: partition-major AND t-block-major so
    the pair-0 half can be DMA'd alone with contiguous lines."""
    return np.ascontiguousarray(
        wT.reshape(CC, P, 2, P).transpose(1, 2, 0, 3).reshape(P, 2 * CC * P))


def _shard_weights(w_qkv, w_proj):
    """Per head-group g: transposed q/k/v weight shards (logically [C, 256]
    with output column order o = 64*h_local + d, stored partition-major)
    and projection shard (logically [256->dup 512, C], partition-major)."""
    shards = []
    for g in range(2):
        heads = range(HG * g, HG * (g + 1))
        q_rows = [h * 3 * HD + d for h in heads for d in range(HD)]
        k_rows = [h * 3 * HD + HD + d for h in heads for d in range(HD)]
        v_rows = [h * 3 * HD + 2 * HD + d for h in heads for d in range(HD)]
        a_chans = [h * HD + (r % HD) for h in heads for r in range(P)]
        shards.append({
            "wqT": _prearrange_qkv(np.ascontiguousarray(w_qkv[q_rows, :].T)),
            "wkT": _prearrange_qkv(np.ascontiguousarray(w_qkv[k_rows, :].T)),
            "wvT": _prearrange_qkv(np.ascontiguousarray(w_qkv[v_rows, :].T)),
            "wpT": _prearrange(np.ascontiguousarray(w_proj[:, a_chans].T), 4),
        })
    return shards


def kernel(x, w_qkv, w_proj, b_proj, _trace=False, _trace_kwargs=None):
    x = np.asarray(x, dtype=np.float32)
    w_qkv = np.asarray(w_qkv, dtype=np.float32)
    w_proj = np.asarray(w_proj, dtype=np.float32)
    b_proj = np.asarray(b_proj, dtype=np.float32)

    if "nc" not in _CACHE:
        _CACHE["nc"] = build_program()
    nc = _CACHE["nc"]

    shards = _shard_weights(w_qkv, w_proj)
    shards = [{k: v.astype(np.float16) for k, v in s.items()} for s in shards]
    in_maps = []
    for core in range(N_CORES):
        b, g = core // 2, core % 2
        m = {"x": np.ascontiguousarray(x[b].astype(np.float16))}
        m.update(shards[g])
        in_maps.append(m)

    kw = {}
    if _trace:
        kw.update(trace=True, trace_cores=[0], **(_trace_kwargs or {}))
    res = run_bass_kernel_spmd(nc, in_maps, list(range(N_CORES)), **kw)

    out = np.empty((B, C, N), dtype=np.float32)
    for b in range(B):
        out[b] = (res.results[2 * b]["out"] + res.results[2 * b + 1]["out"]
                  + b_proj[:, None])
    if _trace:
        _CACHE["last_result"] = res
    return out


# revision 59
# speedup vs baseline: 1.0121x; 1.0021x over previous
"""TRN2 Bass kernel for nn_Attention_47665547051353.

Reference computation (B=4, C=512, N=2048, H=8, hd=64):
    qkv  = w_qkv @ x                           # 1x1 conv
    attn = softmax_j( k^T q * hd^-0.5 )        # softmax over QUERIES j
    out  = w_proj @ (v @ attn) + b_proj
Sharding (8 cores): core c -> batch b = c//2, head-group g = c%2 (4 heads).

Schedule: the exp stream on the Scalar (ACT) engine is the roofline
(128 x ~1.02us ACTIVATE + 64 accumulator reads ~= 143us); everything
else is arranged to keep it saturated end-to-end (~199us @ full clock):
  - head (aggregate-HBM-bound: 8 cores pull inputs at once, ~210GB/s
    each): weights host-pre-arranged partition-major for contiguous DMA
    lines, t-split so only pair-0 halves gate the start; x per-chunk
    tiles on the sync+gpsimd queues (the scalar HWDGE queue is slower);
    PE warm-up matmuls gated on each arriving x chunk keep the HAM at
    2.4GHz through the DMA window; k0/q0 projections interleaved
    cc-wise so unit 0's scores hit PSUM ~1.5us after the last x chunk.
  - steady state: scores double-buffered in PSUM (2x[128,1024]) +
    per-head AV accumulator (4 banks); consecutive matmuls alternate
    PE row/col groups so LDWEIGHTS overlaps and K=64 pairs pack; half-0
    row-sums on a DVE reduce, half-1 on the ACT accumulator (halves
    READ_ACCUMULATOR time; a full DVE offload makes DVE the bottleneck
    and any GpSimd op in the chain adds ~5-7us semaphore latency);
    remaining QKV projections fill every other of the first 17 units,
    their AVs drain afterwards at ~1.5/unit down to a floor of 3.
  - tail (out-BW-bound, 4MB f32/core): the output projection is
    software-pipelined [fill hi0-2 | retire hi3+copy+DMA] across 4 PSUM
    tiles, the first two on the scores banks so the PE rolls straight
    from the last exp into projection matmuls without HAM re-throttle;
    the last head's A-tile copies split Scalar/Vector on different
    banks; half-block copies + DMAs stream on both queues.
"""
import sys

if "/opt/trn_rl_repo" not in sys.path:
    sys.path.insert(0, "/opt/trn_rl_repo")

import numpy as np

import concourse.bass as bass
import concourse.tile as tile
import concourse.mybir as mybir
from concourse import bacc
from concourse.bass_utils import run_bass_kernel_spmd

F32 = mybir.dt.float32
F16 = mybir.dt.float16
EXP = mybir.ActivationFunctionType.Exp

B, C, N = 4, 512, 2048
H, HD = 8, 64
SCALE = HD ** -0.5
P = 128
CC = C // P          # 4 contraction chunks over channels
NT = N // P          # 16 key blocks
HG = H // 2          # 4 heads per core (one head-group)
N_CORES = 8

_CACHE = {}


def build_program(dbg=False):
    nc = bacc.Bacc("TRN2", target_bir_lowering=False, debug=False)
    x_ap = nc.dram_tensor("x", [C, N], F16, kind="ExternalInput").ap()
    # weights arrive host-pre-arranged partition-major so every DMA line
    # is a contiguous 2KB+ read (the naive [C, O] layout forces 512B
    # descriptors, ~5x slower, and the weight loads gate the whole head)
    wq_ap = nc.dram_tensor("wqT", [P, CC * HG * HD], F16, kind="ExternalInput").ap()
    wk_ap = nc.dram_tensor("wkT", [P, CC * HG * HD], F16, kind="ExternalInput").ap()
    wv_ap = nc.dram_tensor("wvT", [P, CC * HG * HD], F16, kind="ExternalInput").ap()
    wp_ap = nc.dram_tensor("wpT", [P, 4 * C], F16, kind="ExternalInput").ap()
    out_ap = nc.dram_tensor("out", [C, N], F32, kind="ExternalOutput").ap()

    with tile.TileContext(nc) as tc:
        with (
            tc.tile_pool(name="const", bufs=1) as const,
            tc.tile_pool(name="big", bufs=1) as big,
            tc.tile_pool(name="ppool", bufs=19) as ppool,
            tc.tile_pool(name="small", bufs=16) as small,
            tc.tile_pool(name="outp", bufs=4) as outp,
        ):
            # ACT exp-table preload (overlaps the input DMAs)
            warm = const.tile([P, 1], F32, tag="warm")
            warm2 = small.tile([P, 1], F32, tag="warm2")
            nc.vector.memset(warm, 0.0)
            nc.scalar.activation(warm2, warm, EXP)

            # scores pool; also reused by the output projection at the tail
            scps_cm = tc.tile_pool(name="scps", bufs=2, space="PSUM")
            scps = scps_cm.__enter__()

            QK = {}
            VT = big.tile([P, NT, HG * HD], F16)
            wp_r = const.tile([P, 4, C], F16)
            A = {}
            units = [(t, h, i) for t in range(2) for h in range(2)
                     for i in range(NT)]
            av_tiles = {}
            pending = []

            def emit_unit(t, h, i, mid=None):
                kt, qt = QK[("k", t)], QK[("q", t)]
                ktd, qtd = QK.get(("kd", t)), QK.get(("qd", t))
                p_t = ppool.tile([P, N], F16, tag="p")
                first_units = (t == 0 and h == 0 and i < 8)
                for half in range(2):
                    if half == 1 and mid is not None:
                        mid()
                    sps = scps.tile([P, 1024], F32, tag="s")
                    for jc in range(2):
                        # alternate PE row groups per matmul so each
                        # LDWEIGHTS overlaps the previous matmul (units 0-7
                        # skip it: the swapped duplicates aren't DMA'd yet
                        # and would stall the in-order PE)
                        if (i + jc) % 2 == 0 or first_units:
                            kk, qq, rb = kt, qt, h * HD
                        else:
                            kk, qq, rb = ktd, qtd, (1 - h) * HD
                        nc.tensor.matmul(
                            sps[:, jc * 512:(jc + 1) * 512],
                            kk[rb:rb + HD, i * P:(i + 1) * P],
                            qq[rb:rb + HD,
                               half * 1024 + jc * 512:half * 1024 + (jc + 1) * 512],
                            start=True, stop=True,
                        )
                    # half 0's row-sum is a DVE reduce over the exp'd fp16
                    # tile (tensor_reduce is ~1.4us/1024 — a full-row
                    # reduce would make DVE the bottleneck); half 1 uses
                    # the ACT accumulator (+182ns READ_ACC).  The add and
                    # reciprocal ride GpSimd (normalize_recip writes
                    # 1/denom back in place).
                    if half == 0:
                        nc.scalar.activation(
                            p_t[:, 0:1024], sps, EXP, scale=SCALE)
                        s0 = small.tile([P, 1], F32, tag="sum0")
                        nc.vector.reduce_sum(s0, p_t[:, 0:1024],
                                             axis=mybir.AxisListType.X)
                    else:
                        s1 = small.tile([P, 1], F32, tag="sum1")
                        nc.scalar.activation(
                            p_t[:, 1024:2048], sps,
                            EXP, scale=SCALE, accum_out=s1)
                s_all = small.tile([P, 1], F32, tag="stot")
                nc.vector.tensor_add(s_all, s0, s1)
                r_t = small.tile([P, 1], F32, tag="rcp")
                nc.vector.reciprocal(r_t, s_all)
                return p_t, r_t

            def emit_av(avps, t, h, i, p_t, r_t):
                vp = small.tile([P, HD], F16, tag="vp")
                hl = 2 * t + h
                nc.vector.tensor_scalar_mul(
                    vp, VT[:, i, hl * HD:(hl + 1) * HD], r_t)
                if (t, h) not in av_tiles:
                    av_new = avps.tile([P, N], F32, tag="av")
                    av_tiles[(t, h)] = av_new
                av = av_tiles[(t, h)]
                for jc4 in range(4):
                    # alternate output col groups per matmul; the halves
                    # are summed by the duplicated projection rows
                    par = (i + jc4) % 2
                    q0 = (par + jc4) % 2
                    nc.tensor.matmul(
                        av[par * HD:(par + 1) * HD,
                           jc4 * 512:(jc4 + 1) * 512],
                        vp,
                        p_t[:, jc4 * 512:(jc4 + 1) * 512],
                        start=(i == q0), stop=(i == NT - 2 + q0),
                        tile_position=(0, par * HD),
                        skip_group_check=True,
                    )
                if i == NT - 1:
                    a_h = big.tile([P, N], F16, tag=f"a{t}{h}")
                    av_done = av_tiles.pop((t, h))
                    last = (t == 1 and h == 1)
                    for q4 in range(4):
                        src = av_done[:, q4 * 512:(q4 + 1) * 512]
                        dst = a_h[:, q4 * 512:(q4 + 1) * 512]
                        if last and q4 < 2:
                            # ACT is idle at this point; DVE+ACT hit
                            # different PSUM banks in parallel
                            nc.scalar.copy(dst, src)
                        else:
                            nc.vector.tensor_copy(dst, src)
                    A[(t, h)] = a_h

            with tc.tile_pool(name="ld", bufs=1) as ld, \
                 tc.tile_pool(name="props", bufs=2, space="PSUM") as props:
                # ---- input DMAs on the sync+gpsimd queues (the scalar
                # HWDGE queue is measurably slower for bulk transfers);
                # gating weights first and t-split so only the pair-0
                # halves (2.25MB total with x) gate the first scores.
                # The head is aggregate-HBM-bound: all 8 cores pull their
                # inputs at once, so per-core BW is ~210GB/s, not 358. ----
                x_view = x_ap.rearrange("(cc p) n -> cc p n", p=P)
                x_r = [ld.tile([P, N], F16, tag=f"x{cc}", name=f"x_r{cc}")
                       for cc in range(CC)]
                wq_r = ld.tile([P, 2, CC, P], F16)
                wk_r = ld.tile([P, 2, CC, P], F16)
                wv_r = ld.tile([P, 2, CC, P], F16)
                wq_v = wq_ap.rearrange("p (t cc o) -> p t cc o", t=2, cc=CC)
                wk_v = wk_ap.rearrange("p (t cc o) -> p t cc o", t=2, cc=CC)
                wv_v = wv_ap.rearrange("p (t cc o) -> p t cc o", t=2, cc=CC)
                nc.sync.dma_start(out=wk_r[:, 0], in_=wk_v[:, 0])
                nc.gpsimd.dma_start(out=wq_r[:, 0], in_=wq_v[:, 0])
                nc.sync.dma_start(out=x_r[0], in_=x_view[0])
                nc.gpsimd.dma_start(out=x_r[1], in_=x_view[1])
                nc.sync.dma_start(out=x_r[2], in_=x_view[2])
                nc.gpsimd.dma_start(out=x_r[3], in_=x_view[3])
                nc.sync.dma_start(out=wk_r[:, 1], in_=wk_v[:, 1])
                nc.gpsimd.dma_start(out=wq_r[:, 1], in_=wq_v[:, 1])
                nc.gpsimd.dma_start(out=wv_r, in_=wv_v)

                # ---- PE warm-up: tiny matmuls ahead of the first real
                # matmul, plus bursts gated on each arriving x chunk (their
                # rhs reads x_r[cc], so each burst fires exactly when its
                # chunk lands) — the HAM stays un-throttled (2.4 GHz)
                # through the whole DMA window by construction.  The
                # scratch PSUM tile is the scores pool's first allocation:
                # spins finish before any unit reuses that buffer. ----
                spin_w = ld.tile([P, P], F16, tag="spin_w")
                nc.vector.memset(spin_w, 0.0)
                spin_ps = scps.tile([P, 1024], F32, tag="s", name="spin_ps")

                def spin(n, rhs=None):
                    r = spin_w[:, 0:P] if rhs is None else rhs
                    for _ in range(n):
                        nc.tensor.matmul(spin_ps[:, 0:P], spin_w, r,
                                         start=True, stop=True)

                spin(14)

                def w_chunk(w_r, t, cc):
                    return w_r[:, t, cc, :]

                def emit_qk_chunk(wname, w_r, t, half, cast_on=None):
                    """One [128,1024] output chunk of a q/k projection."""
                    key = (wname, t)
                    if key not in QK:
                        dst_new = big.tile([P, N], F16, tag=f"{wname}{t}")
                        QK[key] = dst_new
                    dst = QK[key]
                    ps = props.tile([P, 1024], F32, tag="qk")
                    for cc in range(CC):
                        for jc in range(2):
                            j0 = jc * 512
                            nc.tensor.matmul(
                                ps[:, j0:j0 + 512],
                                w_chunk(w_r, t, cc),
                                x_r[cc][:, half * 1024 + j0:half * 1024 + j0 + 512],
                                start=(cc == 0), stop=(cc == CC - 1),
                            )
                    dslice = dst[:, half * 1024:(half + 1) * 1024]
                    if cast_on is nc.scalar:
                        nc.scalar.copy(dslice, ps)
                    else:
                        nc.vector.tensor_copy(dslice, ps)
                    if half == 1:
                        dstd = big.tile([P, N], F16, tag=f"{wname}d{t}")
                        # dups ride gpsimd: the sync queue is busy with the
                        # VT transposes, and scalar-queue DMAs steal ACT
                        # time mid-stream
                        nc.gpsimd.dma_start(out=dstd[0:HD, :], in_=dst[HD:2 * HD, :])
                        nc.gpsimd.dma_start(out=dstd[HD:2 * HD, :], in_=dst[0:HD, :])
                        QK[(wname + "d", t)] = dstd

                # pair-0 K and Q interleaved cc-wise (two PSUM buffers) so
                # after the last x chunk lands only ~4 matmuls + the casts
                # remain before unit 0's scores; q0h0's cast runs on the
                # idle Scalar engine in parallel with k0h0's on Vector.
                k0_key, q0_key = ("k", 0), ("q", 0)
                QK[k0_key] = big.tile([P, N], F16, tag="k0", name="k0t")
                QK[q0_key] = big.tile([P, N], F16, tag="q0", name="q0t")
                ps_k = props.tile([P, 1024], F32, tag="qk", name="ps_k")
                ps_q = props.tile([P, 1024], F32, tag="qk", name="ps_q")
                for cc in range(CC):
                    if cc < CC - 1:
                        spin(8, rhs=x_r[cc][:, 0:P])
                    for ps, w_r in ((ps_k, wk_r), (ps_q, wq_r)):
                        for jc in range(2):
                            j0 = jc * 512
                            nc.tensor.matmul(
                                ps[:, j0:j0 + 512],
                                w_chunk(w_r, 0, cc),
                                x_r[cc][:, j0:j0 + 512],
                                start=(cc == 0), stop=(cc == CC - 1),
                            )
                nc.vector.tensor_copy(QK[k0_key][:, 0:1024], ps_k)
                nc.scalar.copy(QK[q0_key][:, 0:1024], ps_q)

                def emit_v_chunk(vt2, half, vr):
                    ps = props.tile([P, 1024], F32, tag="qk")
                    for cc in range(CC):
                        for jc in range(2):
                            j0 = jc * 512
                            nc.tensor.matmul(
                                ps[:, j0:j0 + 512],
                                w_chunk(wv_r, vt2, cc),
                                x_r[cc][:, half * 1024 + j0:half * 1024 + j0 + 512],
                                start=(cc == 0), stop=(cc == CC - 1),
                            )
                    nc.vector.tensor_copy(vr[:, half * 1024:(half + 1) * 1024], ps)
                    if half == 1:
                        # transpose DMAs are HWDGE-only; keep them all on
                        # the sync queue (plenty of slack before first use)
                        for nt in range(NT):
                            nc.sync.dma_start(
                                out=VT[:, nt, vt2 * P:(vt2 + 1) * P],
                                in_=vr[:, nt * P:(nt + 1) * P],
                                transpose=True,
                            )

                # v projections / VT transposes / pair-1 Q/K spread over
                # every other unit of the first 17 (one fill chunk is
                # ~1.7us of PE time vs ~2.2us of ACT time per unit, so
                # half a fill per unit keeps the exp stream PE-limited
                # never ACT-starved); their AVs drain afterwards at
                # ~1.5/unit.  Deadlines: k0h1 before unit 8's QK, q1/k1
                # dups before unit 32's QK (~t94us), VT vt2=1 before the
                # first pair-1 AV drain (~t112us).
                vrow0 = ld.tile([P, N], F16, tag="vrow0")
                vrow1 = ld.tile([P, N], F16, tag="vrow1")
                fill_at = {
                    1: lambda: emit_qk_chunk("k", wk_r, 0, 1),
                    2: lambda: emit_v_chunk(0, 0, vrow0),
                    4: lambda: emit_v_chunk(0, 1, vrow0),
                    6: lambda: emit_qk_chunk("q", wq_r, 1, 0),
                    8: lambda: emit_qk_chunk("q", wq_r, 1, 1),
                    10: lambda: emit_qk_chunk("k", wk_r, 1, 0),
                    12: lambda: emit_qk_chunk("k", wk_r, 1, 1),
                    14: lambda: emit_v_chunk(1, 0, vrow1),
                    16: lambda: emit_v_chunk(1, 1, vrow1),
                }
                n_pre = 17

                # wp is only needed by the tail projection; keep it off the
                # critical input-bandwidth window
                nc.gpsimd.dma_start(out=wp_r, in_=wp_ap.rearrange("p (t o) -> p t o", t=4))

                # unit 0's half-0 scores run as soon as k0h0+q0h0 exist;
                # q0h1 is projected between its two halves so the first
                # ACTIVATE fires while q0's second half still computes
                for g in range(n_pre):
                    u = units[g]
                    if g == 0:
                        mid = lambda: emit_qk_chunk("q", wq_r, 0, 1)
                    else:
                        mid = None
                    pending.append((u, emit_unit(*u, mid=mid)))
                    if g in fill_at:
                        fill_at[g]()

            # ---- main attention stream (software-pipelined) ----
            with tc.tile_pool(name="avps", bufs=1, space="PSUM") as avps:
                for g in range(n_pre, len(units)):
                    u = units[g]
                    pending.append((u, emit_unit(*u)))
                    # drain the fill-region backlog at ~1.5 AVs per unit
                    # (PE stays under the ACT period), down to a floor of
                    # 3 (~7us of exp runway) so the head-transition A-tile
                    # copies never starve the ACT stream
                    drain_to = max(4, n_pre - (g - n_pre + 1) // 2 * 3 + 2)
                    while len(pending) > drain_to:
                        (pt_, ph_, pi_), (p_t, r_t) = pending.pop(0)
                        emit_av(avps, pt_, ph_, pi_, p_t, r_t)
                while pending:
                    (pt_, ph_, pi_), (p_t, r_t) = pending.pop(0)
                    emit_av(avps, pt_, ph_, pi_, p_t, r_t)

            # ---- output projection, in a 3-deep PSUM pool over the banks
            # freed by scores+AV (its first tile lands on the ex-scores
            # banks, so the matmuls start the moment the last exp
            # finishes); hi emitted done-heads-first so the in-order PE
            # only waits on the last head at the very end; hi-cast on the
            # idle Scalar engine, fp16 residual on Vector ----
            out_q = [nc.sync, nc.gpsimd]
            with tc.tile_pool(name="prx", bufs=2, space="PSUM") as prx:
                def proj_fill(blk):
                    """hi 0-2 matmuls of a block (A(1,1)-independent).
                    Blocks 0-1 use the scores pool's buffers (free the
                    moment the last exp ends — so the PE rolls straight
                    from attention into the projection and stays warm);
                    later blocks rotate through the ex-AV banks."""
                    ot, hf = blk // 2, blk % 2
                    if blk % 4 < 2:
                        pso = scps.tile([P, 1024], F32, tag="s")
                    else:
                        pso = prx.tile([P, 1024], F32, tag="pr")
                    base = hf * 1024
                    for hi in range(3):
                        t2, h2 = hi // 2, hi % 2
                        for jc in range(2):
                            nc.tensor.matmul(
                                pso[:, jc * 512:(jc + 1) * 512],
                                wp_r[:, hi, ot * P:(ot + 1) * P],
                                A[(t2, h2)][:, base + jc * 512:base + (jc + 1) * 512],
                                start=(hi == 0), stop=False,
                            )
                    return pso

                def proj_retire(blk, pso):
                    """last-head matmuls + split-float copies + DMAs."""
                    ot, hf = blk // 2, blk % 2
                    base = hf * 1024
                    for jc in range(2):
                        nc.tensor.matmul(
                            pso[:, jc * 512:(jc + 1) * 512],
                            wp_r[:, 3, ot * P:(ot + 1) * P],
                            A[(1, 1)][:, base + jc * 512:base + (jc + 1) * 512],
                            start=False, stop=True,
                        )
                    o_sb = outp.tile([P, 1024], F32, tag="o")
                    # half-granular copy+DMA: transfers start ~0.7us
                    # earlier and stream on both queues concurrently
                    for q4 in range(2):
                        sl = slice(q4 * 512, (q4 + 1) * 512)
                        nc.vector.tensor_copy(o_sb[:, sl], pso[:, sl])
                        out_q[(blk + q4) % 2].dma_start(
                            out=out_ap[ot * P:(ot + 1) * P,
                                       base + q4 * 512:base + (q4 + 1) * 512],
                            in_=o_sb[:, sl])

                # 3-block software pipeline: the first blocks' hi 0-2
                # matmuls bridge the A(1,1) copy latency so the PE never
                # idles (and never HAM-re-throttles) through the tail
                inflight = []
                for blk in range(8):
                    inflight.append((blk, proj_fill(blk)))
                    if len(inflight) == 3:
                        proj_retire(*inflight.pop(0))
                while inflight:
                    proj_retire(*inflight.pop(0))

            scps_cm.__exit__(None, None, None)

    nc.compile()
    return nc


def _prearrange(wT, chunks):
    """[chunks*128, O] -> [128, chunks*O]: partition-major layout so every
    DMA line is one contiguous read per partition."""
    O = wT.shape[1]
    return np.ascontiguousarray(
        wT.reshape(chunks, P, O).transpose(1, 0, 2).reshape(P, chunks * O))


def _prearrange_qkv(wT):
    """[C, 256] -> [128, 2*CC*128]: partition-major AND t-block-major so
    the pair-0 half can be DMA'd alone with contiguous lines."""
    return np.ascontiguousarray(
        wT.reshape(CC, P, 2, P).transpose(1, 2, 0, 3).reshape(P, 2 * CC * P))


def _shard_weights(w_qkv, w_proj):
    """Per head-group g: transposed q/k/v weight shards (logically [C, 256]
    with output column order o = 64*h_local + d, stored partition-major)
    and projection shard (logically [256->dup 512, C], partition-major)."""
    shards = []
    for g in range(2):
        heads = range(HG * g, HG * (g + 1))
        q_rows = [h * 3 * HD + d for h in heads for d in range(HD)]
        k_rows = [h * 3 * HD + HD + d for h in heads for d in range(HD)]
        v_rows = [h * 3 * HD + 2 * HD + d for h in heads for d in range(HD)]
        a_chans = [h * HD + (r % HD) for h in heads for r in range(P)]
        shards.append({
            "wqT": _prearrange_qkv(np.ascontiguousarray(w_qkv[q_rows, :].T)),
            "wkT": _prearrange_qkv(np.ascontiguousarray(w_qkv[k_rows, :].T)),
            "wvT": _prearrange_qkv(np.ascontiguousarray(w_qkv[v_rows, :].T)),
            "wpT": _prearrange(np.ascontiguousarray(w_proj[:, a_chans].T), 4),
        })
    return shards


def kernel(x, w_qkv, w_proj, b_proj, _trace=False, _trace_kwargs=None):
    x = np.asarray(x, dtype=np.float32)
    w_qkv = np.asarray(w_qkv, dtype=np.float32)
    w_proj = np.asarray(w_proj, dtype=np.float32)
    b_proj = np.asarray(b_proj, dtype=np.float32)

    if "nc" not in _CACHE:
        _CACHE["nc"] = build_program()
    nc = _CACHE["nc"]

    shards = _shard_weights(w_qkv, w_proj)
    shards = [{k: v.astype(np.float16) for k, v in s.items()} for s in shards]
    in_maps = []
    for core in range(N_CORES):
        b, g = core // 2, core % 2
        m = {"x": np.ascontiguousarray(x[b].astype(np.float16))}
        m.update(shards[g])
        in_maps.append(m)

    kw = {}
    if _trace:
        kw.update(trace=True, trace_cores=[0], **(_trace_kwargs or {}))
    res = run_bass_kernel_spmd(nc, in_maps, list(range(N_CORES)), **kw)

    out = np.empty((B, C, N), dtype=np.float32)
    for b in range(B):
        out[b] = (res.results[2 * b]["out"] + res.results[2 * b + 1]["out"]
                  + b_proj[:, None])
    if _trace:
        _CACHE["last_result"] = res
    return out
